# revision 1
# baseline (speedup 1.0000x reference)
"""Trainium2 Bass kernel for nn_CachedAttention (B=4, T=2048, D=2048, H=16, start_pos=0).

Sharding: 8 cores = 4 batches x 2 head-groups. Core i handles batch i//2 and
heads (i%2)*8 .. (i%2)*8+8. Each core computes QKV projections for its heads,
causal attention, and a partial output projection (its heads' contribution to
the full output, in bf16). The host sums the two partials per batch.

Structure (all matmul operands bf16, fp32 PSUM accumulation):
- V is projected tok-major directly (stationary = x.T tile, moving = Wv
  strip), one PSUM bank per head during the V phase, so no PE transposes.
- Scores are built transposed (S.T[kpos, qpos]) in PSUM groups of up to 3
  kpos-tiles ([128, 1536] f32 = 3 banks) so ONE activation instruction
  exps up to 1536 columns, amortizing ACT's ~350-cycle per-instruction
  overhead (the baseline's per-tile exp made attention ACT-bound).
- Causal diagonal tiles are narrowed: kpos-tile j of chunk c only needs
  qpos >= 128j, so QK/PV matmuls and exp shrink to 512-128j columns.
  A single [128,512] step mask (m[p,o]=p<=o) zeroes the in-tile triangle.
- Softmax denominators: probability tiles are tree-summed on DVE (bf16)
  into an f32 chunk accumulator; one ones-vector matmul per chunk does the
  final partition-dim reduction on the PE.
- Output y is evacuated unnormalized (frees the PSUM bank early) and
  normalized in place with 1/colsum broadcast via a DRAM bounce.
"""
import math

import numpy as np
import ml_dtypes

import concourse.bass as bass
import concourse.tile as tile
from concourse import mybir
from concourse.bass_utils import run_bass_kernel_spmd
from concourse.vector_clock import ScopedClock

bf16 = mybir.dt.bfloat16
f32 = mybir.dt.float32

B, T, D, H = 4, 2048, 2048, 16
DK = D // H          # 128
HL = H // 2          # heads per core = 8
FT = D // 128        # feature tiles = 16
TT = T // 128        # token tiles = 16
NC_CHUNK = 512       # qpos chunk
NCH = T // NC_CHUNK  # 4 chunks
SCALE = 1.0 / math.sqrt(DK)
N_CORES = 8


# ---------------------------------------------------------------------------
# Workaround: this toolchain's walrus rejects Drain instructions that carry
# attached sem waits ("Too many sync wait commands"). Emit the global-clock
# waits as standalone wait_ge instructions instead, then a bare drain.
# ---------------------------------------------------------------------------
def _patched_drain_and_barrier(self, tick_clock, wait_clock):
    nop = self.nc.sync.nop()
    wait_clock.add_sem_waits(nop.ins, ScopedClock({None: tick_clock.global_clock}))
    si = nop.ins.sync_info
    waits = list(si.on_wait or []) if si is not None else []
    if si is not None and waits:
        si.on_wait = []
    handles = {h.num: h for h in self.sems.allocated().values()}
    for w in waits:
        assert w.wait_mode == "sem-ge-imm", w
        h = handles.get(w.id)
        assert h is not None, f"no handle for sem id {w.id} ({w.ant_name})"
        self.nc.sync.wait_ge(h, w.wait_value)
    self.nc.sync.drain()
    self.nc.all_engine_barrier(sem_only=True)
    assert self.sems is not None
    popped = self.nc._tile_sem_poison_stack.pop()
    assert popped is self._sem_poison
    self.nc.clear_and_free_semaphores(list(self.sems.allocated().values()))
    self.nc.all_engine_barrier(sem_only=True)


def _apply_tile_patch():
    tile.TileContext._drain_and_barrier = _patched_drain_and_barrier


def _hoist_excess_waits(nc, cap=1):
    """Walrus in this toolchain fits at most `cap` attached sem-waits per
    instruction (0 for InstISA). Hoist extras into standalone
    InstEventSemaphore waits emitted immediately before, on the same engine."""
    import bass_rust
    for f in nc.m.functions:
        for blk in f.blocks:
            new = []
            for inst in blk.instructions:
                si = inst.sync_info
                ow = list(si.on_wait) if si is not None and si.on_wait else []
                my_cap = 0 if type(inst).__name__ == "InstISA" else cap
                if len(ow) > my_cap:
                    keep = ow[:my_cap]
                    hoist = ow[my_cap:]
                    for k, w in enumerate(hoist):
                        ev = mybir.InstEventSemaphore(
                            name=f"{inst.name}-w{k}",
                            engine=inst.engine,
                            ins=[],
                            outs=[],
                            sync_info=bass_rust.SyncInfo(
                                on_wait=[w], on_update=[]),
                        )
                        nc.register_instruction(ev)
                        new.append(ev)
                    si.on_wait = keep
                new.append(inst)
            blk.instructions = new


# diagonal-group packing: per chunk, kpos-tile 4c+j only needs qpos>=128j.
# Column offsets inside the 3-bank [128,1536] f32 group tile keep every
# matmul output within one PSUM bank and the exp reads contiguous.
DIAG_OFF = (0, 512, 1024, 1280)
DIAG_W = (512, 384, 256, 128)


def _full_groups(nfull):
    g = [3] * (nfull // 3)
    if nfull % 3:
        g.append(nfull % 3)
    return g


# ---------------------------------------------------------------------------
# Device program (identical on all 8 cores; per-core data comes via in_maps)
# ---------------------------------------------------------------------------
def build_program(reps=1, device_loop=False):
    _apply_tile_patch()
    nc = bass.Bass()

    xT_d = nc.dram_tensor("xT", [FT, 128, T], bf16, kind="ExternalInput")
    wq_d = nc.dram_tensor("wq", [HL, 128, FT, DK], bf16, kind="ExternalInput")
    wk_d = nc.dram_tensor("wk", [HL, 128, FT, DK], bf16, kind="ExternalInput")
    wv_d = nc.dram_tensor("wv", [FT, 128, HL * DK], bf16, kind="ExternalInput")
    wo_d = nc.dram_tensor("wo", [HL, 128, D], bf16, kind="ExternalInput")
    mask_d = nc.dram_tensor("mask", [128, NC_CHUNK], bf16, kind="ExternalInput")
    out_d = nc.dram_tensor("out", [T, D], bf16, kind="ExternalOutput")
    rb_d = nc.dram_tensor("rb", [HL, NCH, NC_CHUNK], f32)  # recip bounce

    from contextlib import ExitStack
    with tile.TileContext(nc) as tc:
        with ExitStack() as stack:
            ep = lambda p: stack.enter_context(p)
            xt_pool = ep(tc.tile_pool(name="xt", bufs=FT))
            # wv strips early, wo strips late — same tag shares the slots
            wv_pool = ep(tc.tile_pool(name="wv", bufs=FT))
            wqk_pool = ep(tc.tile_pool(name="wqk", bufs=1))
            qk_pool = ep(tc.tile_pool(name="qk", bufs=2))
            v_pool = ep(tc.tile_pool(name="vsb", bufs=TT))
            y_pool = ep(tc.tile_pool(name="yt", bufs=HL))
            pt_pool = ep(tc.tile_pool(name="pt", bufs=3))
            small_pool = ep(tc.tile_pool(name="small", bufs=2))
            rc_pool = ep(tc.tile_pool(name="rcp", bufs=1))
            o_pool = ep(tc.tile_pool(name="ost", bufs=2))
            const_pool = ep(tc.tile_pool(name="const", bufs=1))
            ps_st = ep(tc.tile_pool(name="ps_st", bufs=1, space="PSUM"))
            ps_ot = ep(tc.tile_pool(name="ps_ot", bufs=2, space="PSUM"))
            ps_dn = ep(tc.tile_pool(name="ps_dn", bufs=1, space="PSUM"))
            ps_mm = ep(tc.tile_pool(name="ps_mm", bufs=2, space="PSUM"))
            wo_pool = wv_pool
            # constants
            ones = const_pool.tile([128, 1], bf16, tag="ones")
            nc.vector.memset(ones[:], 1.0)
            mask_sb = const_pool.tile([128, NC_CHUNK], bf16, tag="mask")
            nc.sync.dma_start(out=mask_sb[:], in_=mask_d[:, :])

            # resident x.T tiles (feature-major); chunk-major DMA order so the
            # first qpos chunk of every feature tile lands first
            xt_sb = []
            for _f in range(FT):
                xt_tile = xt_pool.tile([128, T], bf16, tag="xt")
                xt_sb.append(xt_tile)
            for c in range(NCH):
                for f in range(FT):
                    nc.sync.dma_start(
                        out=xt_sb[f][:, c * NC_CHUNK:(c + 1) * NC_CHUNK],
                        in_=xT_d[f][:, c * NC_CHUNK:(c + 1) * NC_CHUNK],
                    )

            import contextlib
            rep_iter = (
                [None] if device_loop else range(reps))
            for _rep in rep_iter:
              with (tc.For_i(0, reps, 1, hint_engines=(mybir.EngineType.PE, mybir.EngineType.DVE, mybir.EngineType.Activation, mybir.EngineType.SP)) if device_loop
                    else contextlib.nullcontext()):
                def emit_v_proj():
                    # moving operand = Wv.T strip covering 4 heads' dk dims
                    # concatenated, so each PSUM bank accumulates one
                    # [tok, 512] tile as a single accumulation group.
                    wvt_sb = []
                    for f in range(FT):
                        wvt = wv_pool.tile([128, HL * DK], bf16, tag="wv",
                                           name=f"wvt{f}")
                        nc.scalar.dma_start(out=wvt[:], in_=wv_d[f])
                        wvt_sb.append(wvt)
                    v_sb = []
                    for tt in range(TT):
                        vpa = ps_mm.tile([128, 512], f32, tag="mm",
                                         name=f"vpa{tt}")
                        vpb = ps_mm.tile([128, 512], f32, tag="mm",
                                         name=f"vpb{tt}")
                        for f in range(FT):
                            xs = xt_sb[f][:, tt * 128:(tt + 1) * 128]
                            nc.tensor.matmul(
                                vpa[:], xs, wvt_sb[f][:, 0:512],
                                start=(f == 0), stop=(f == FT - 1),
                                skip_group_check=True,
                            )
                            nc.tensor.matmul(
                                vpb[:], xs, wvt_sb[f][:, 512:1024],
                                start=(f == 0), stop=(f == FT - 1),
                                skip_group_check=True,
                            )
                        vt = v_pool.tile([128, HL * DK], bf16, tag="v",
                                         name=f"vt{tt}")
                        nc.scalar.copy(out=vt[:, 0:512], in_=vpa[:])
                        nc.scalar.copy(out=vt[:, 512:1024], in_=vpb[:])
                        v_sb.append(vt)
                    return v_sb

                def emit_qk_proj(h):
                    wq_s = wqk_pool.tile([128, FT, DK], bf16, tag="wq")
                    nc.scalar.dma_start(out=wq_s[:], in_=wq_d[h])
                    wk_s = wqk_pool.tile([128, FT, DK], bf16, tag="wk")
                    nc.scalar.dma_start(out=wk_s[:], in_=wk_d[h])
                    qT_s = qk_pool.tile([128, T], bf16, tag="qT")
                    kT_s = qk_pool.tile([128, T], bf16, tag="kT")
                    for w_s, dst in ((wq_s, qT_s), (wk_s, kT_s)):
                        for half in range(2):
                            c0, c1 = 2 * half, 2 * half + 1
                            psa = ps_mm.tile([128, NC_CHUNK], f32, tag="mm")
                            psb = ps_mm.tile([128, NC_CHUNK], f32, tag="mm")
                            for f in range(FT):
                                nc.tensor.matmul(
                                    psa[:], w_s[:, f, :],
                                    xt_sb[f][:, c0 * NC_CHUNK:(c0 + 1) * NC_CHUNK],
                                    start=(f == 0), stop=(f == FT - 1),
                                    skip_group_check=True,
                                )
                                nc.tensor.matmul(
                                    psb[:], w_s[:, f, :],
                                    xt_sb[f][:, c1 * NC_CHUNK:(c1 + 1) * NC_CHUNK],
                                    start=(f == 0), stop=(f == FT - 1),
                                    skip_group_check=True,
                                )
                            nc.scalar.copy(
                                out=dst[:, c0 * NC_CHUNK:(c0 + 1) * NC_CHUNK],
                                in_=psa[:])
                            nc.scalar.copy(
                                out=dst[:, c1 * NC_CHUNK:(c1 + 1) * NC_CHUNK],
                                in_=psb[:])
                    return qT_s, kT_s

                yt_sb = []

                def emit_attention(h, qT_s, kT_s):
                    yT_s = y_pool.tile([128, T], bf16, tag="yt")
                    yt_sb.append(yT_s)
                    hs = slice(h * DK, (h + 1) * DK)
                    for c in range(NCH):
                        qs = qT_s[:, c * NC_CHUNK:(c + 1) * NC_CHUNK]
                        ot = ps_ot.tile([128, NC_CHUNK], f32, tag="ot")
                        dn = ps_dn.tile([1, NC_CHUNK], f32, tag="dn")
                        first_pv = True
                        first_dn = True
                        t0 = 0
                        for L in _full_groups(4 * c):
                            ts = list(range(t0, t0 + L))
                            t0 += L
                            st = ps_st.tile([128, 1536], f32, tag="st")
                            for j, t in enumerate(ts):
                                nc.tensor.matmul(
                                    st[:, j * 512:(j + 1) * 512],
                                    kT_s[:, t * 128:(t + 1) * 128], qs,
                                    start=True, stop=True,
                                    skip_group_check=True,
                                )
                            pt = pt_pool.tile([128, 1536], bf16, tag="pt")
                            nc.scalar.activation(
                                out=pt[:, :L * 512], in_=st[:, :L * 512],
                                func=mybir.ActivationFunctionType.Exp,
                                scale=SCALE,
                            )
                            for j, t in enumerate(ts):
                                nc.tensor.matmul(
                                    ot[:], v_sb[t][:, hs],
                                    pt[:, j * 512:(j + 1) * 512],
                                    start=first_pv, stop=False,
                                    skip_group_check=True,
                                )
                                first_pv = False
                            # denominator: ones-MM per tile, accumulated
                            # in the dn PSUM bank (partition-dim reduce)
                            for j in range(L):
                                nc.tensor.matmul(
                                    dn[:], ones[:],
                                    pt[:, j * 512:(j + 1) * 512],
                                    start=first_dn, stop=False,
                                    skip_group_check=True,
                                )
                                first_dn = False
                        # --- diagonal group (narrowed, packed in 3 banks) ---
                        st = ps_st.tile([128, 1536], f32, tag="st")
                        pt = pt_pool.tile([128, 1536], bf16, tag="pt")
                        for j in range(4):
                            t = 4 * c + j
                            off, w = DIAG_OFF[j], DIAG_W[j]
                            nc.tensor.matmul(
                                st[:, off:off + w],
                                kT_s[:, t * 128:(t + 1) * 128],
                                qT_s[:, c * NC_CHUNK + 128 * j:(c + 1) * NC_CHUNK],
                                start=True, stop=True, skip_group_check=True,
                            )
                        nc.scalar.activation(
                            out=pt[:, 0:896], in_=st[:, 0:896],
                            func=mybir.ActivationFunctionType.Exp, scale=SCALE)
                        nc.scalar.activation(
                            out=pt[:, 1024:1408], in_=st[:, 1024:1408],
                            func=mybir.ActivationFunctionType.Exp, scale=SCALE)
                        for j in range(4):
                            off, w = DIAG_OFF[j], DIAG_W[j]
                            nc.vector.tensor_mul(
                                pt[:, off:off + w], pt[:, off:off + w],
                                mask_sb[:, 0:w])
                        for j in range(4):
                            t = 4 * c + j
                            off, w = DIAG_OFF[j], DIAG_W[j]
                            nc.tensor.matmul(
                                ot[:, 128 * j:NC_CHUNK], v_sb[t][:, hs],
                                pt[:, off:off + w],
                                start=first_pv, stop=(j == 3),
                                skip_group_check=True,
                            )
                            first_pv = False
                        ys = yT_s[:, c * NC_CHUNK:(c + 1) * NC_CHUNK]
                        nc.scalar.copy(out=ys, in_=ot[:])
                        for j in range(4):
                            off, w = DIAG_OFF[j], DIAG_W[j]
                            nc.tensor.matmul(
                                dn[:, 128 * j:NC_CHUNK], ones[:],
                                pt[:, off:off + w],
                                start=first_dn, stop=(j == 3),
                                skip_group_check=True,
                            )
                            first_dn = False
                        rc = rc_pool.tile([1, NC_CHUNK], f32, tag="rc")
                        nc.vector.reciprocal(rc[:], dn[:])
                        nc.sync.dma_start(out=rb_d[h, c], in_=rc[:])
                        bc = small_pool.tile([128, NC_CHUNK], f32, tag="bc")
                        r_ap = rb_d[h, c]
                        bcast = bass.AP(
                            tensor=r_ap.tensor, offset=r_ap.offset,
                            ap=[[0, 128]] + list(r_ap.ap),
                        )
                        nc.sync.dma_start(out=bc[:], in_=bcast)
                        nc.vector.tensor_mul(ys, ys, bc[:])

                qT_s, kT_s = emit_qk_proj(0)
                v_sb = emit_v_proj()
                wo_strips = []
                for h in range(HL):
                    emit_attention(h, qT_s, kT_s)
                    if h + 1 < HL:
                        qT_s, kT_s = emit_qk_proj(h + 1)
                    if h == HL - 1:
                        # prefetch Wo strips (xT slots are draining by now)
                        for half in range(2):
                            for hh in range(HL):
                                ws = wo_pool.tile([128, 1024], bf16, tag="wv")
                                nc.scalar.dma_start(
                                    out=ws[:],
                                    in_=wo_d[hh][:, half * 1024:(half + 1) * 1024],
                                )
                                wo_strips.append(ws)

                # --- output projection ---
                for half in range(2):
                    for tt in range(TT):
                        poa = ps_mm.tile([128, NC_CHUNK], f32, tag="mm")
                        pob = ps_mm.tile([128, NC_CHUNK], f32, tag="mm")
                        for h in range(HL):
                            ws = wo_strips[half * HL + h]
                            yt_ap = yt_sb[h][:, tt * 128:(tt + 1) * 128]
                            nc.tensor.matmul(
                                poa[:], yt_ap, ws[:, 0:512],
                                start=(h == 0), stop=(h == HL - 1),
                                skip_group_check=True,
                            )
                            nc.tensor.matmul(
                                pob[:], yt_ap, ws[:, 512:1024],
                                start=(h == 0), stop=(h == HL - 1),
                                skip_group_check=True,
                            )
                        for k, po in ((0, poa), (1, pob)):
                            dc = 2 * half + k
                            o_s = o_pool.tile([128, NC_CHUNK], bf16, tag="o")
                            nc.scalar.copy(out=o_s[:], in_=po[:])
                            nc.sync.dma_start(
                                out=out_d[tt * 128:(tt + 1) * 128,
                                          dc * NC_CHUNK:(dc + 1) * NC_CHUNK],
                                in_=o_s[:],
                            )

    _hoist_excess_waits(nc)
    nc.finalize()
    return nc


_NC_CACHE = {}


def get_program(reps=1, device_loop=False):
    key = (reps, device_loop)
    if key not in _NC_CACHE:
        _NC_CACHE[key] = build_program(reps, device_loop)
    return _NC_CACHE[key]


# ---------------------------------------------------------------------------
# Host-side sharding / layout prep
# ---------------------------------------------------------------------------
def _step_mask():
    kp = np.arange(128)[:, None]
    qp = np.arange(NC_CHUNK)[None, :]
    return (kp <= qp).astype(np.float32).astype(ml_dtypes.bfloat16)


def _w_strips(w_loc):
    # w_loc: [1024 out-dims, 2048 feat] -> [h, p(feat within tile), f, j(dk)]
    a = np.ascontiguousarray(w_loc.reshape(HL, DK, FT, 128).transpose(0, 3, 2, 1))
    return a.astype(ml_dtypes.bfloat16)


def make_in_maps(x, Wq, Wk, Wv, Wo):
    mask = _step_mask()
    in_maps = []
    for core in range(N_CORES):
        b, hg = core // 2, core % 2
        sl = slice(hg * HL * DK, (hg + 1) * HL * DK)
        xT = np.ascontiguousarray(x[b].T).reshape(FT, 128, T)
        wo_loc = np.ascontiguousarray(Wo[:, sl].T).reshape(HL, 128, D)
        in_maps.append({
            "xT": xT.astype(ml_dtypes.bfloat16),
            "wq": _w_strips(Wq[sl, :]),
            "wk": _w_strips(Wk[sl, :]),
            "wv": np.ascontiguousarray(
                Wv[sl, :].T.reshape(FT, 128, HL * DK)).astype(
                    ml_dtypes.bfloat16),
            "wo": wo_loc.astype(ml_dtypes.bfloat16),
            "mask": mask,
        })
    return in_maps


def combine(results):
    out = np.empty((B, T, D), dtype=np.float32)
    for b in range(B):
        out[b] = (results[2 * b]["out"].astype(np.float32)
                  + results[2 * b + 1]["out"].astype(np.float32))
    return out


def kernel(x, Wq, Wk, Wv, Wo, k_cache, v_cache, start_pos, **_ignored):
    x = np.asarray(x, dtype=np.float32)
    Wq = np.asarray(Wq, dtype=np.float32)
    Wk = np.asarray(Wk, dtype=np.float32)
    Wv = np.asarray(Wv, dtype=np.float32)
    Wo = np.asarray(Wo, dtype=np.float32)
    assert int(start_pos) == 0, "kernel specialized for start_pos=0 prefill"
    assert x.shape == (B, T, D)

    nc = get_program()
    in_maps = make_in_maps(x, Wq, Wk, Wv, Wo)
    res = run_bass_kernel_spmd(nc, in_maps, list(range(N_CORES)))
    return combine(res.results)


if __name__ == "__main__":
    rng = np.random.default_rng(0)
    x = rng.standard_normal((B, T, D)).astype(np.float32)
    mk = lambda: (rng.standard_normal((D, D)) * 0.02).astype(np.float32)
    out = kernel(x, mk(), mk(), mk(), mk(),
                 np.zeros((B, H, T, DK), np.float32),
                 np.zeros((B, H, T, DK), np.float32), 0)
    print(out.shape, out.dtype, np.abs(out).max())

